# revision 8
# baseline (speedup 1.0000x reference)
"""Trainium2 Bass kernel for nn_AttentionAggregator3d.

Math (per batch b):
    zmf = zm.reshape(CM, N)                     # N = D*W*H = 4096 tokens
    q = Wq @ zmf + bq ; k = Wk @ zmf + bk       # (16, N)
    v = Wv @ zmf + bv                           # (128, N)
    A = softmax_n(q^T k)                        # (N, N), softmax over keys n
    out = v @ A^T ; result = zc + gamma * out

Kernel structure (v2, ScalarE-bound design):
  * logits^T[n, m] = zm_n^T G^T zm_m with G = Wq^T Wk folded on host; the
    query-side transform tq = G zm_q (128 x 1024) is computed ONCE and used
    as the bf16 MOVING operand of every logits matmul, with the key chunk
    zm_j as the stationary.  (The baseline materialized t = G zm over all
    4096 keys and burned 8 ScalarE copies staging it; ScalarE is the
    bottleneck engine, so those copies came straight out of the critical
    path.)
  * All matmul operands are bf16 (host pre-converts zm/G/Wv^T); exp output
    E is bf16 too, which (a) halves SBUF traffic and (b) lets the DVE
    denominator accumulation run in the 2x/4x 16-bit perf modes.
  * Softmax denominators: per-chunk partition sums are split between PE
    (ones-matmul accumulating into one PSUM bank), DVE and GPSIMD bf16
    accumulators, tuned so no engine exceeds the ScalarE exp stream
    (32 x ~1.04us, the roofline of this kernel).
  * Sharding: 8 cores = batch (2) x query-block (4, 1024 queries each),
    zm rolled per core so its query block sits at columns 0:1024.
  * ACT tables (exp+ln set) are prefetched with a dummy exp at t=0 so the
    ~2.7us table load overlaps the input DMAs.
"""

import os
import sys
import types

import ml_dtypes
import numpy as np

import concourse.bacc as bacc_mod
import concourse.tile as tile
from concourse import mybir
from concourse.bass_utils import run_bass_kernel_spmd

B, CC, CM, P = 2, 128, 128, 16
N = 16 * 16 * 16          # 4096 tokens
MBLK = N // 4             # 1024 queries per core
NCORES = 8
NCHUNK = N // 128         # 32 key chunks of 128

F32 = mybir.dt.float32
F32R = mybir.dt.float32r
BF16 = mybir.dt.bfloat16
AF = mybir.ActivationFunctionType
ALU = mybir.AluOpType

LAST_RESULTS = None  # BassKernelResults of the most recent run (for test.py)


def _ensure_ntff_hook() -> bool:
    """The grading image lacks antenv.axon_hooks; synthesize it from the
    boot module's ctypes NTFF driver so trace=True works under axon."""
    try:
        import antenv.axon_hooks  # noqa: F401

        return True
    except ImportError:
        pass
    try:
        import antenv
        from trn_agent_boot.trn_boot import _ntff_profile_via_ctypes

        hook = _ntff_profile_via_ctypes("/opt/axon/libaxon_pjrt.so")
        mod = types.ModuleType("antenv.axon_hooks")
        mod.get_axon_ntff_profile_hook = lambda: hook
        mod.set_axon_ntff_profile_hook = lambda h: None
        sys.modules["antenv.axon_hooks"] = mod
        antenv.axon_hooks = mod
        return hook is not None
    except Exception:
        return False


# Route Exp and Ln to the one table set that holds both, so the kernel pays a
# single ACT_TABLE_LOAD instead of three (exp -> ln -> exp again).
_orig_gat = bacc_mod.get_activation_tables
_COMBINED_SET = "natural_log_exp_and_others"


def _patched_gat(arch):
    tabs = _orig_gat(arch)
    if _COMBINED_SET in tabs:
        for name, fns in tabs.items():
            if name != _COMBINED_SET:
                fns.discard(AF.Exp)
                fns.discard(AF.Ln)
    return tabs


bacc_mod.get_activation_tables = _patched_gat


def _build(use_qk_bias: bool):
    nc = bacc_mod.Bacc(
        "TRN2",
        target_bir_lowering=False,
        debug=False,
        num_devices=NCORES,
    )

    zm_d = nc.dram_tensor("zm", (CM, N), BF16, kind="ExternalInput").ap()
    zc_d = nc.dram_tensor("zc", (CC, MBLK), F32, kind="ExternalInput").ap()
    gt_d = nc.dram_tensor("gt", (CM, CM), BF16, kind="ExternalInput").ap()
    wvt_d = nc.dram_tensor("wvt", (CM, CC), BF16, kind="ExternalInput").ap()
    # packed per-partition scalars: col 0 = gamma, col 1 = gamma*bv
    sc_d = nc.dram_tensor("sc", (CC, 2), F32, kind="ExternalInput").ap()
    onesc_d = nc.dram_tensor("onesc", (128, 1), BF16, kind="ExternalInput").ap()
    onesr_d = nc.dram_tensor("onesr", (1, 128), BF16, kind="ExternalInput").ap()
    if use_qk_bias:
        u_d = nc.dram_tensor("u", (CM, 1), BF16, kind="ExternalInput").ap()
    out_d = nc.dram_tensor("out", (CC, MBLK), F32, kind="ExternalOutput").ap()

    # denominator routing (see module docstring): half 0 of chunk j goes to
    # the PE ones-matmul when j % 4 == 0, else to the DVE acc0 accumulator;
    # half 1 goes to GPSIMD when j % 3 == 1, else to the DVE acc accumulator.
    PE_H0 = [j for j in range(NCHUNK) if j % 4 == 0]
    GP_H1 = [j for j in range(NCHUNK) if j % 3 == 1]

    with tile.TileContext(nc) as tc:
        with (
            tc.tile_pool(name="consts", bufs=1) as consts,
            tc.tile_pool(name="epool", bufs=6) as epool,
            tc.tile_pool(name="lpool", bufs=2, space="PSUM") as lpool,
            tc.tile_pool(name="stage", bufs=1, space="PSUM") as stage,
            tc.tile_pool(name="opool", bufs=1, space="PSUM") as opool,
            tc.tile_pool(name="spool", bufs=1, space="PSUM") as spool,
        ):
            zm_bf = consts.tile([CM, N], BF16, tag="zm")
            tq_bf = consts.tile([CM, MBLK], BF16, tag="tq")
            vt_bf = consts.tile([128, N], BF16, tag="vt")  # chunk j at cols 128j
            zc_sb = consts.tile([CC, MBLK], F32, tag="zc")
            gt_sb = consts.tile([CM, CM], BF16, tag="gt")
            wvt_sb = consts.tile([CM, CC], BF16, tag="wvt")
            sc_sb = consts.tile([CC, 2], F32, tag="sc")
            ones_col = consts.tile([128, 1], BF16, tag="onesc")
            ones_row = consts.tile([1, 128], BF16, tag="onesr")
            acc0 = consts.tile([128, 512], BF16, tag="acc0")   # DVE, half 0
            acc = consts.tile([128, 512], BF16, tag="acc")     # DVE, half 1
            accg = consts.tile([128, 512], BF16, tag="accg")   # GPSIMD, half 1
            lns = consts.tile([1, MBLK], F32, tag="lns")
            rvec = consts.tile([1, MBLK], BF16, tag="rvec")
            rb_sb = consts.tile([128, MBLK], F32, tag="rb")
            tmp_sb = consts.tile([CC, MBLK], F32, tag="tmp")
            out_sb = consts.tile([CC, MBLK], F32, tag="outsb")
            scr = consts.tile([1, 1], F32, tag="scr")
            if use_qk_bias:
                u_sb = consts.tile([CM, 1], BF16, tag="u")
                rn_sb = consts.tile([128, NCHUNK], F32, tag="rn")

            # ---- input DMAs, fanned across idle engine sequencers; the
            # ones vector goes first so the ACT table prefetch below can
            # start immediately ----
            nc.sync.dma_start(ones_col[:], onesc_d)

            # ACT table prefetch: a dummy exp with no real consumers makes
            # ScalarE pay the ~2.7us exp/ln table load during the input DMAs
            # instead of in front of the first real exp
            nc.scalar.activation(scr[0:1, 0:1], ones_col[0:1, 0:1], AF.Exp)

            nc.gpsimd.dma_start(ones_row[:], onesr_d)
            nc.sync.dma_start(gt_sb[:], gt_d)
            nc.sync.dma_start(zm_bf[:, 0:1024], zm_d[:, 0:1024])
            nc.scalar.dma_start(zm_bf[:, 1024:2048], zm_d[:, 1024:2048])
            nc.gpsimd.dma_start(zm_bf[:, 2048:3072], zm_d[:, 2048:3072])
            nc.gpsimd.dma_start(zm_bf[:, 3072:4096], zm_d[:, 3072:4096])
            nc.sync.dma_start(wvt_sb[:], wvt_d)
            nc.sync.dma_start(sc_sb[:], sc_d)
            if use_qk_bias:
                nc.gpsimd.dma_start(u_sb[:], u_d)
            nc.sync.dma_start(zc_sb[:], zc_d)

            gam_ap = sc_sb[:, 0:1]
            adv_ap = sc_sb[:, 1:2]

            out_ps = opool.tile([CC, MBLK], F32, tag="out")
            s_ps = spool.tile([1, 512], F32, tag="s")  # PE denom, half 0

            # ---- tq = G zm_q over the core's 1024 query columns ----
            tq_ps = lpool.tile([128, MBLK], F32, tag="L")
            for h in range(2):
                nc.tensor.matmul(
                    tq_ps[:, h * 512 : (h + 1) * 512],
                    gt_sb[:],
                    zm_bf[:, h * 512 : (h + 1) * 512],
                    start=True,
                    stop=True,
                )
            nc.vector.tensor_copy(tq_bf[:], tq_ps[:])

            def emit_vt_batch(i):
                # vt chunk j = (zm chunk j)^T @ Wv^T for j in 4i..4i+3
                vps = stage.tile([128, 512], F32, tag="S")
                for k in range(4):
                    j = 4 * i + k
                    nc.tensor.matmul(
                        vps[:, 128 * k : 128 * (k + 1)],
                        zm_bf[:, 128 * j : 128 * (j + 1)],
                        wvt_sb[:],
                        start=True,
                        stop=True,
                    )
                nc.vector.tensor_copy(vt_bf[:, i * 512 : (i + 1) * 512], vps[:])
                if use_qk_bias:
                    rnps = stage.tile([128, 512], F32, tag="S")
                    for k in range(4):
                        j = 4 * i + k
                        nc.tensor.matmul(
                            rnps[:, k : k + 1],
                            zm_bf[:, 128 * j : 128 * (j + 1)],
                            u_sb[:],
                            start=True,
                            stop=True,
                        )
                    nc.vector.tensor_copy(
                        rn_sb[:, 4 * i : 4 * (i + 1)], rnps[:, 0:4]
                    )

            emit_vt_batch(0)

            e_tiles = {}
            pe_h0 = 0  # ones-matmuls emitted into the s_ps accumulation group

            LAG = int(os.environ.get("BASS_PV_LAG", "3"))
            for j in range(NCHUNK + LAG):
                if j < NCHUNK:
                    if j % 4 == 2 and j // 4 + 1 <= 7:
                        emit_vt_batch(j // 4 + 1)
                    # logits^T chunk j: (keys 128, queries 1024)
                    lps = lpool.tile([128, MBLK], F32, tag="L")
                    for h in range(2):
                        nc.tensor.matmul(
                            lps[:, h * 512 : (h + 1) * 512],
                            zm_bf[:, 128 * j : 128 * (j + 1)],
                            tq_bf[:, h * 512 : (h + 1) * 512],
                            start=True,
                            stop=True,
                        )
                    ej = epool.tile([128, MBLK], BF16, tag="E")
                    bias = rn_sb[:, j : j + 1] if use_qk_bias else 0.0
                    nc.scalar.activation(ej[:], lps[:], AF.Exp, bias=bias)
                    e_tiles[j] = ej
                    # softmax-denominator accumulation for chunk j
                    if j in PE_H0:
                        nc.tensor.matmul(
                            s_ps[0:1, :],
                            ones_col[:],
                            ej[:, 0:512],
                            start=(pe_h0 == 0),
                            stop=False,
                            skip_group_check=True,
                        )
                        pe_h0 += 1
                    else:
                        if j == 1:
                            nc.vector.tensor_copy(acc0[:], ej[:, 0:512])
                        else:
                            nc.vector.tensor_add(acc0[:], acc0[:], ej[:, 0:512])
                    if j in GP_H1:
                        if j == GP_H1[0]:
                            nc.gpsimd.tensor_copy(accg[:], ej[:, 512:1024])
                        else:
                            nc.gpsimd.tensor_add(accg[:], accg[:], ej[:, 512:1024])
                    else:
                        if j == 0:
                            nc.vector.tensor_copy(acc[:], ej[:, 512:1024])
                        else:
                            nc.vector.tensor_add(acc[:], acc[:], ej[:, 512:1024])
                if j >= LAG:
                    jj = j - LAG
                    ej = e_tiles.pop(jj)
                    for h in range(2):
                        nc.tensor.matmul(
                            out_ps[:, h * 512 : (h + 1) * 512],
                            vt_bf[:, 128 * jj : 128 * (jj + 1)],
                            ej[:, h * 512 : (h + 1) * 512],
                            start=(jj == 0),
                            stop=(jj == NCHUNK - 1),
                        )

            # fold the DVE half-0 accumulator into the PE PSUM sums
            nc.tensor.matmul(
                s_ps[0:1, :],
                ones_col[:],
                acc0[:],
                start=False,
                stop=True,
                skip_group_check=True,
            )

            # tail in 256-wide quarters so the ln/exp/broadcast/final/DMA
            # chains of successive quarters overlap across engines
            for q in range(4):
                sl = slice(q * 256, (q + 1) * 256)
                if q < 2:
                    s_src = s_ps[0:1, q * 256 : (q + 1) * 256]
                else:
                    # fold the DVE + GPSIMD half-1 accumulators
                    sfold = stage.tile([128, 512], F32, tag="S")
                    qs = slice((q - 2) * 256, (q - 1) * 256)
                    nc.tensor.matmul(
                        sfold[0:1, 0:256], ones_col[:], acc[:, qs],
                        start=True, stop=False,
                    )
                    nc.tensor.matmul(
                        sfold[0:1, 0:256], ones_col[:], accg[:, qs],
                        start=False, stop=True,
                    )
                    s_src = sfold[0:1, 0:256]
                # r = 1/s via exp(-ln s): same ACT table set as the main exps
                nc.scalar.activation(lns[:, sl], s_src, AF.Ln)
                nc.scalar.activation(rvec[:, sl], lns[:, sl], AF.Exp, scale=-1.0)
                # broadcast r across partitions with a K=1 matmul, fold gamma
                rb_ps = stage.tile([128, 512], F32, tag="S")
                nc.tensor.matmul(
                    rb_ps[:, 0:256], ones_row[:], rvec[:, sl],
                    start=True, stop=True,
                )
                nc.vector.tensor_scalar(
                    out=rb_sb[:, sl],
                    in0=rb_ps[:, 0:256],
                    scalar1=gam_ap,
                    scalar2=None,
                    op0=ALU.mult,
                )
                # out = zc + (outPV * gamma/s + gamma*bv)
                nc.vector.tensor_tensor(
                    tmp_sb[:, sl], out_ps[:, sl], rb_sb[:, sl], op=ALU.mult
                )
                nc.vector.scalar_tensor_tensor(
                    out_sb[:, sl],
                    tmp_sb[:, sl],
                    adv_ap,
                    zc_sb[:, sl],
                    op0=ALU.add,
                    op1=ALU.add,
                )
                nc.sync.dma_start(out_d[:, sl], out_sb[:, sl])

    nc.compile()
    return nc


_CACHE = {}


def _get_program(use_qk_bias: bool):
    if use_qk_bias not in _CACHE:
        _CACHE[use_qk_bias] = _build(use_qk_bias)
    return _CACHE[use_qk_bias]


def kernel(zc, zm, Wq, bq, Wk, bk, Wv, bv, gamma):
    global LAST_RESULTS
    zc = np.ascontiguousarray(zc, dtype=np.float32)
    zmf = np.asarray(zm, dtype=np.float32).reshape(B, CM, N)
    zmf_bf = zmf.astype(ml_dtypes.bfloat16)
    zcf = zc.reshape(B, CC, N)

    Wq = np.asarray(Wq, dtype=np.float32)
    Wk = np.asarray(Wk, dtype=np.float32)
    Wv = np.asarray(Wv, dtype=np.float32)
    gt = (Wk.astype(np.float64).T @ Wq.astype(np.float64)).astype(
        ml_dtypes.bfloat16
    )
    wvt = np.ascontiguousarray(Wv.T).astype(ml_dtypes.bfloat16)
    gamma_v = np.float32(np.asarray(gamma).reshape(-1)[0])
    sc_arr = np.empty((CC, 2), dtype=np.float32)
    sc_arr[:, 0] = gamma_v
    sc_arr[:, 1] = gamma_v * np.asarray(bv, dtype=np.float32)

    use_qk_bias = bool(np.any(bq)) or bool(np.any(bk))
    nc = _get_program(use_qk_bias)

    in_maps = []
    for c in range(NCORES):
        b, jblk = divmod(c, 4)
        m = {
            "zm": np.ascontiguousarray(
                np.roll(zmf_bf[b], -MBLK * jblk, axis=1)
            ),
            "zc": np.ascontiguousarray(zcf[b][:, MBLK * jblk : MBLK * (jblk + 1)]),
            "gt": gt,
            "wvt": wvt,
            "sc": sc_arr,
            "onesc": np.ones((128, 1), dtype=ml_dtypes.bfloat16),
            "onesr": np.ones((1, 128), dtype=ml_dtypes.bfloat16),
        }
        if use_qk_bias:
            m["u"] = np.ascontiguousarray(
                (Wk.T @ np.asarray(bq, dtype=np.float32)).reshape(CM, 1)
            ).astype(ml_dtypes.bfloat16)
        in_maps.append(m)

    trace = bool(int(os.environ.get("BASS_KERNEL_TRACE", "0")))
    if trace and not _ensure_ntff_hook():
        trace = False
    res = run_bass_kernel_spmd(
        nc,
        in_maps,
        core_ids=list(range(NCORES)),
        trace=trace,
    )
    LAST_RESULTS = res

    out = np.empty((B, CC, N), dtype=np.float32)
    for c in range(NCORES):
        b, jblk = divmod(c, 4)
        out[b][:, MBLK * jblk : MBLK * (jblk + 1)] = res.results[c]["out"]
    return out.reshape(zc.shape)


# revision 10
# speedup vs baseline: 1.0120x; 1.0120x over previous
"""Trainium2 Bass kernel for nn_AttentionAggregator3d.

Math (per batch b):
    zmf = zm.reshape(CM, N)                     # N = D*W*H = 4096 tokens
    q = Wq @ zmf + bq ; k = Wk @ zmf + bk       # (16, N)
    v = Wv @ zmf + bv                           # (128, N)
    A = softmax_n(q^T k)                        # (N, N), softmax over keys n
    out = v @ A^T ; result = zc + gamma * out

Kernel structure (v2, ScalarE-bound design):
  * logits^T[n, m] = zm_n^T G^T zm_m with G = Wq^T Wk folded on host; the
    query-side transform tq = G zm_q (128 x 1024) is computed ONCE and used
    as the bf16 MOVING operand of every logits matmul, with the key chunk
    zm_j as the stationary.  (The baseline materialized t = G zm over all
    4096 keys and burned 8 ScalarE copies staging it; ScalarE is the
    bottleneck engine, so those copies came straight out of the critical
    path.)
  * All matmul operands are bf16 (host pre-converts zm/G/Wv^T); exp output
    E is bf16 too, which (a) halves SBUF traffic and (b) lets the DVE
    denominator accumulation run in the 2x/4x 16-bit perf modes.
  * Softmax denominators: per-chunk partition sums are split between PE
    (ones-matmul accumulating into one PSUM bank), DVE and GPSIMD bf16
    accumulators, tuned so no engine exceeds the ScalarE exp stream
    (32 x ~1.04us, the roofline of this kernel).
  * Sharding: 8 cores = batch (2) x query-block (4, 1024 queries each),
    zm rolled per core so its query block sits at columns 0:1024.
  * ACT tables (exp+ln set) are prefetched with a dummy exp at t=0 so the
    ~2.7us table load overlaps the input DMAs.
"""

import os
import sys
import types

import ml_dtypes
import numpy as np

import concourse.bacc as bacc_mod
import concourse.tile as tile
from concourse import mybir
from concourse.bass_utils import run_bass_kernel_spmd

B, CC, CM, P = 2, 128, 128, 16
N = 16 * 16 * 16          # 4096 tokens
MBLK = N // 4             # 1024 queries per core
NCORES = 8
NCHUNK = N // 128         # 32 key chunks of 128

F32 = mybir.dt.float32
F32R = mybir.dt.float32r
BF16 = mybir.dt.bfloat16
AF = mybir.ActivationFunctionType
ALU = mybir.AluOpType

LAST_RESULTS = None  # BassKernelResults of the most recent run (for test.py)


def _ensure_ntff_hook() -> bool:
    """The grading image lacks antenv.axon_hooks; synthesize it from the
    boot module's ctypes NTFF driver so trace=True works under axon."""
    try:
        import antenv.axon_hooks  # noqa: F401

        return True
    except ImportError:
        pass
    try:
        import antenv
        from trn_agent_boot.trn_boot import _ntff_profile_via_ctypes

        hook = _ntff_profile_via_ctypes("/opt/axon/libaxon_pjrt.so")
        mod = types.ModuleType("antenv.axon_hooks")
        mod.get_axon_ntff_profile_hook = lambda: hook
        mod.set_axon_ntff_profile_hook = lambda h: None
        sys.modules["antenv.axon_hooks"] = mod
        antenv.axon_hooks = mod
        return hook is not None
    except Exception:
        return False


# Route Exp and Ln to the one table set that holds both, so the kernel pays a
# single ACT_TABLE_LOAD instead of three (exp -> ln -> exp again).
_orig_gat = bacc_mod.get_activation_tables
_COMBINED_SET = "natural_log_exp_and_others"


def _patched_gat(arch):
    tabs = _orig_gat(arch)
    if _COMBINED_SET in tabs:
        for name, fns in tabs.items():
            if name != _COMBINED_SET:
                fns.discard(AF.Exp)
                fns.discard(AF.Ln)
    return tabs


bacc_mod.get_activation_tables = _patched_gat


def _build(use_qk_bias: bool):
    nc = bacc_mod.Bacc(
        "TRN2",
        target_bir_lowering=False,
        debug=False,
        num_devices=NCORES,
    )

    zm_d = nc.dram_tensor("zm", (CM, N), BF16, kind="ExternalInput").ap()
    zc_d = nc.dram_tensor("zc", (CC, MBLK), F32, kind="ExternalInput").ap()
    gt_d = nc.dram_tensor("gt", (CM, CM), BF16, kind="ExternalInput").ap()
    wvt_d = nc.dram_tensor("wvt", (CM, CC), BF16, kind="ExternalInput").ap()
    # packed per-partition scalars: col 0 = gamma, col 1 = gamma*bv
    sc_d = nc.dram_tensor("sc", (CC, 2), F32, kind="ExternalInput").ap()
    onesc_d = nc.dram_tensor("onesc", (128, 1), BF16, kind="ExternalInput").ap()
    onesr_d = nc.dram_tensor("onesr", (1, 128), BF16, kind="ExternalInput").ap()
    if use_qk_bias:
        u_d = nc.dram_tensor("u", (CM, 1), BF16, kind="ExternalInput").ap()
    out_d = nc.dram_tensor("out", (CC, MBLK), F32, kind="ExternalOutput").ap()

    # denominator routing (see module docstring): half 0 of chunk j goes to
    # the PE ones-matmul when j % 4 == 0, else to the DVE acc0 accumulator;
    # half 1 goes to GPSIMD when j % 3 == 1, else to the DVE acc accumulator.
    PE_H0 = [j for j in range(NCHUNK) if j % 4 == 0]
    GP_H1 = [j for j in range(NCHUNK) if j % 3 == 1]

    with tile.TileContext(nc) as tc:
        with (
            tc.tile_pool(name="consts", bufs=1) as consts,
            tc.tile_pool(name="epool", bufs=6) as epool,
            tc.tile_pool(name="lpool", bufs=2, space="PSUM") as lpool,
            tc.tile_pool(name="stage", bufs=1, space="PSUM") as stage,
            tc.tile_pool(name="opool", bufs=1, space="PSUM") as opool,
            tc.tile_pool(name="spool", bufs=1, space="PSUM") as spool,
        ):
            zm_bf = consts.tile([CM, N], BF16, tag="zm")
            tq_bf = consts.tile([CM, MBLK], BF16, tag="tq")
            vt_bf = consts.tile([128, N], BF16, tag="vt")  # chunk j at cols 128j
            zc_sb = consts.tile([CC, MBLK], F32, tag="zc")
            gt_sb = consts.tile([CM, CM], BF16, tag="gt")
            wvt_sb = consts.tile([CM, CC], BF16, tag="wvt")
            sc_sb = consts.tile([CC, 2], F32, tag="sc")
            ones_col = consts.tile([128, 1], BF16, tag="onesc")
            ones_row = consts.tile([1, 128], BF16, tag="onesr")
            acc0 = consts.tile([128, 512], BF16, tag="acc0")   # DVE, half 0
            acc = consts.tile([128, 512], BF16, tag="acc")     # DVE, half 1
            accg = consts.tile([128, 512], BF16, tag="accg")   # GPSIMD, half 1
            lns = consts.tile([1, MBLK], F32, tag="lns")
            rvec = consts.tile([1, MBLK], BF16, tag="rvec")
            rb_sb = consts.tile([128, MBLK], F32, tag="rb")
            tmp_sb = consts.tile([CC, MBLK], F32, tag="tmp")
            out_sb = consts.tile([CC, MBLK], F32, tag="outsb")
            scr = consts.tile([1, 1], F32, tag="scr")
            if use_qk_bias:
                u_sb = consts.tile([CM, 1], BF16, tag="u")
                rn_sb = consts.tile([128, NCHUNK], F32, tag="rn")

            # ---- input DMAs, fanned across idle engine sequencers; the
            # ones vector goes first so the ACT table prefetch below can
            # start immediately ----
            nc.sync.dma_start(ones_col[:], onesc_d)

            # ACT table prefetch: a dummy exp with no real consumers makes
            # ScalarE pay the ~2.7us exp/ln table load during the input DMAs
            # instead of in front of the first real exp
            nc.scalar.activation(scr[0:1, 0:1], ones_col[0:1, 0:1], AF.Exp)

            nc.gpsimd.dma_start(ones_row[:], onesr_d)
            nc.sync.dma_start(gt_sb[:], gt_d)
            nc.sync.dma_start(zm_bf[:, 0:1024], zm_d[:, 0:1024])
            nc.scalar.dma_start(zm_bf[:, 1024:2048], zm_d[:, 1024:2048])
            nc.gpsimd.dma_start(zm_bf[:, 2048:3072], zm_d[:, 2048:3072])
            nc.gpsimd.dma_start(zm_bf[:, 3072:4096], zm_d[:, 3072:4096])
            nc.sync.dma_start(wvt_sb[:], wvt_d)
            nc.sync.dma_start(sc_sb[:], sc_d)
            if use_qk_bias:
                nc.gpsimd.dma_start(u_sb[:], u_d)
            nc.sync.dma_start(zc_sb[:], zc_d)

            gam_ap = sc_sb[:, 0:1]
            adv_ap = sc_sb[:, 1:2]

            out_ps = opool.tile([CC, MBLK], F32, tag="out")
            s_ps = spool.tile([1, 512], F32, tag="s")  # PE denom, half 0

            # ---- tq = G zm_q over the core's 1024 query columns ----
            tq_ps = lpool.tile([128, MBLK], F32, tag="L")
            for h in range(2):
                nc.tensor.matmul(
                    tq_ps[:, h * 512 : (h + 1) * 512],
                    gt_sb[:],
                    zm_bf[:, h * 512 : (h + 1) * 512],
                    start=True,
                    stop=True,
                )
            nc.vector.tensor_copy(tq_bf[:], tq_ps[:])

            def emit_vt_batch(i):
                # vt chunk j = (zm chunk j)^T @ Wv^T for j in 4i..4i+3
                vps = stage.tile([128, 512], F32, tag="S")
                for k in range(4):
                    j = 4 * i + k
                    nc.tensor.matmul(
                        vps[:, 128 * k : 128 * (k + 1)],
                        zm_bf[:, 128 * j : 128 * (j + 1)],
                        wvt_sb[:],
                        start=True,
                        stop=True,
                    )
                nc.vector.tensor_copy(vt_bf[:, i * 512 : (i + 1) * 512], vps[:])
                if use_qk_bias:
                    rnps = stage.tile([128, 512], F32, tag="S")
                    for k in range(4):
                        j = 4 * i + k
                        nc.tensor.matmul(
                            rnps[:, k : k + 1],
                            zm_bf[:, 128 * j : 128 * (j + 1)],
                            u_sb[:],
                            start=True,
                            stop=True,
                        )
                    nc.vector.tensor_copy(
                        rn_sb[:, 4 * i : 4 * (i + 1)], rnps[:, 0:4]
                    )

            emit_vt_batch(0)

            e_tiles = {}
            pe_h0 = 0  # ones-matmuls emitted into the s_ps accumulation group

            LAG = int(os.environ.get("BASS_PV_LAG", "3"))
            for j in range(NCHUNK + LAG):
                if j < NCHUNK:
                    if j % 4 == 2 and j // 4 + 1 <= 7:
                        emit_vt_batch(j // 4 + 1)
                    # logits^T chunk j: (keys 128, queries 1024)
                    lps = lpool.tile([128, MBLK], F32, tag="L")
                    for h in range(2):
                        nc.tensor.matmul(
                            lps[:, h * 512 : (h + 1) * 512],
                            zm_bf[:, 128 * j : 128 * (j + 1)],
                            tq_bf[:, h * 512 : (h + 1) * 512],
                            start=True,
                            stop=True,
                        )
                    ej = epool.tile([128, MBLK], BF16, tag="E")
                    bias = rn_sb[:, j : j + 1] if use_qk_bias else 0.0
                    nc.scalar.activation(ej[:], lps[:], AF.Exp, bias=bias)
                    e_tiles[j] = ej
                    # softmax-denominator accumulation for chunk j
                    if j in PE_H0:
                        nc.tensor.matmul(
                            s_ps[0:1, :],
                            ones_col[:],
                            ej[:, 0:512],
                            start=(pe_h0 == 0),
                            stop=False,
                            skip_group_check=True,
                        )
                        pe_h0 += 1
                    else:
                        if j == 1:
                            nc.vector.tensor_copy(acc0[:], ej[:, 0:512])
                        else:
                            nc.vector.tensor_add(acc0[:], acc0[:], ej[:, 0:512])
                    if j in GP_H1:
                        if j == GP_H1[0]:
                            nc.gpsimd.tensor_copy(accg[:], ej[:, 512:1024])
                        else:
                            nc.gpsimd.tensor_add(accg[:], accg[:], ej[:, 512:1024])
                    else:
                        if j == 0:
                            nc.vector.tensor_copy(acc[:], ej[:, 512:1024])
                        else:
                            nc.vector.tensor_add(acc[:], acc[:], ej[:, 512:1024])
                if j >= LAG:
                    jj = j - LAG
                    ej = e_tiles.pop(jj)
                    for h in range(2):
                        nc.tensor.matmul(
                            out_ps[:, h * 512 : (h + 1) * 512],
                            vt_bf[:, 128 * jj : 128 * (jj + 1)],
                            ej[:, h * 512 : (h + 1) * 512],
                            start=(jj == 0),
                            stop=(jj == NCHUNK - 1),
                        )

            # fold the DVE half-0 accumulator into the PE PSUM sums
            nc.tensor.matmul(
                s_ps[0:1, :],
                ones_col[:],
                acc0[:],
                start=False,
                stop=True,
                skip_group_check=True,
            )

            # tail in 256-wide quarters so the ln/exp/broadcast/final/DMA
            # chains of successive quarters overlap across engines
            for q in range(4):
                sl = slice(q * 256, (q + 1) * 256)
                if q < 2:
                    s_src = s_ps[0:1, q * 256 : (q + 1) * 256]
                else:
                    # fold the DVE + GPSIMD half-1 accumulators
                    sfold = stage.tile([128, 512], F32, tag="S")
                    qs = slice((q - 2) * 256, (q - 1) * 256)
                    nc.tensor.matmul(
                        sfold[0:1, 0:256], ones_col[:], acc[:, qs],
                        start=True, stop=False,
                    )
                    nc.tensor.matmul(
                        sfold[0:1, 0:256], ones_col[:], accg[:, qs],
                        start=False, stop=True,
                    )
                    s_src = sfold[0:1, 0:256]
                # r = 1/s via exp(-ln s): same ACT table set as the main exps
                nc.scalar.activation(lns[:, sl], s_src, AF.Ln)
                nc.scalar.activation(rvec[:, sl], lns[:, sl], AF.Exp, scale=-1.0)
                # broadcast r across partitions with a K=1 matmul, fold gamma
                rb_ps = stage.tile([128, 512], F32, tag="S")
                nc.tensor.matmul(
                    rb_ps[:, 0:256], ones_row[:], rvec[:, sl],
                    start=True, stop=True,
                )
                nc.vector.tensor_scalar(
                    out=rb_sb[:, sl],
                    in0=rb_ps[:, 0:256],
                    scalar1=gam_ap,
                    scalar2=None,
                    op0=ALU.mult,
                )
                # out = zc + (outPV * gamma/s + gamma*bv)
                nc.vector.tensor_tensor(
                    tmp_sb[:, sl], out_ps[:, sl], rb_sb[:, sl], op=ALU.mult
                )
                nc.vector.scalar_tensor_tensor(
                    out_sb[:, sl],
                    tmp_sb[:, sl],
                    adv_ap,
                    zc_sb[:, sl],
                    op0=ALU.add,
                    op1=ALU.add,
                )
                nc.sync.dma_start(out_d[:, sl], out_sb[:, sl])

    nc.compile()
    return nc


_CACHE = {}


def _get_program(use_qk_bias: bool):
    if use_qk_bias not in _CACHE:
        _CACHE[use_qk_bias] = _build(use_qk_bias)
    return _CACHE[use_qk_bias]


def kernel(zc, zm, Wq, bq, Wk, bk, Wv, bv, gamma):
    global LAST_RESULTS
    zc = np.ascontiguousarray(zc, dtype=np.float32)
    zmf = np.asarray(zm, dtype=np.float32).reshape(B, CM, N)
    zmf_bf = zmf.astype(ml_dtypes.bfloat16)
    zcf = zc.reshape(B, CC, N)

    Wq = np.asarray(Wq, dtype=np.float32)
    Wk = np.asarray(Wk, dtype=np.float32)
    Wv = np.asarray(Wv, dtype=np.float32)
    # lps[n,m] = sum_c zm[c,n] tq[c,m] must equal k_n . q_m = zm_n^T (Wk^T Wq) zm_m,
    # so tq = (Wk^T Wq) zm_q; the tq matmul computes gt^T @ zm_q, hence
    # gt = (Wk^T Wq)^T = Wq^T Wk.
    gt = (Wq.astype(np.float64).T @ Wk.astype(np.float64)).astype(
        ml_dtypes.bfloat16
    )
    wvt = np.ascontiguousarray(Wv.T).astype(ml_dtypes.bfloat16)
    gamma_v = np.float32(np.asarray(gamma).reshape(-1)[0])
    sc_arr = np.empty((CC, 2), dtype=np.float32)
    sc_arr[:, 0] = gamma_v
    sc_arr[:, 1] = gamma_v * np.asarray(bv, dtype=np.float32)

    use_qk_bias = bool(np.any(bq)) or bool(np.any(bk))
    nc = _get_program(use_qk_bias)

    in_maps = []
    for c in range(NCORES):
        b, jblk = divmod(c, 4)
        m = {
            "zm": np.ascontiguousarray(
                np.roll(zmf_bf[b], -MBLK * jblk, axis=1)
            ),
            "zc": np.ascontiguousarray(zcf[b][:, MBLK * jblk : MBLK * (jblk + 1)]),
            "gt": gt,
            "wvt": wvt,
            "sc": sc_arr,
            "onesc": np.ones((128, 1), dtype=ml_dtypes.bfloat16),
            "onesr": np.ones((1, 128), dtype=ml_dtypes.bfloat16),
        }
        if use_qk_bias:
            m["u"] = np.ascontiguousarray(
                (Wk.T @ np.asarray(bq, dtype=np.float32)).reshape(CM, 1)
            ).astype(ml_dtypes.bfloat16)
        in_maps.append(m)

    trace = bool(int(os.environ.get("BASS_KERNEL_TRACE", "0")))
    if trace and not _ensure_ntff_hook():
        trace = False
    res = run_bass_kernel_spmd(
        nc,
        in_maps,
        core_ids=list(range(NCORES)),
        trace=trace,
    )
    LAST_RESULTS = res

    out = np.empty((B, CC, N), dtype=np.float32)
    for c in range(NCORES):
        b, jblk = divmod(c, 4)
        out[b][:, MBLK * jblk : MBLK * (jblk + 1)] = res.results[c]["out"]
    return out.reshape(zc.shape)
